# revision 1
# baseline (speedup 1.0000x reference)
"""Self-contained Trainium2 Bass kernel for a Transformer encoder layer.

Reference computation (fp32):
    q,k,v = x@wq, x@wk, x@wv          (per-head split, DK=64)
    attn  = softmax(q@k^T/sqrt(DK) + mask_bias) @ v
    x1    = LN(x + attn@wo) * g1 + be1
    out   = LN(x1 + relu(x1@w1 + b1)@w2 + b2) * g2 + be2

Sharding: pure data-parallel over (batch, seq). 8 cores; core c owns batch
c//4 and a 1024-row query shard (c%4). K/V projections for the full batch
are computed redundantly on each core (no collectives).

On-core dataflow (per core):
  - x^T slices built via PE transpose; K^T (pair-packed, [128,4096] per
    2-head pair) spilled to DRAM scratch; V kept resident with a ones
    column appended per head (rowsum of softmax comes out of the ctx
    matmul for free).
  - scores computed transposed S^T[k, q]; each psum tile holds one
    query-half for both heads of a pair so the two score matmuls use
    disjoint PE row groups (0-63 / 64-127) and run concurrently; exp on
    ACT with the mask bias as a per-partition activation bias; softmax
    normalization deferred through the V-matmul and applied after the
    attention phase via a partition-broadcast DMA of the rowsum row
    plus a DVE reciprocal+multiply, feeding pair-contracted wo matmuls.
  - FFN computed via h1^T = relu(w1^T @ x1^T + b1) so the dff bias is a
    per-partition activation bias; second matmul contracts h1^T chunks.
  - All matmuls run as float32r (reduced-precision fp32, full PE rate).
"""

import os
import sys

import numpy as np

if os.path.isdir("/opt/trn_rl_repo") and "/opt/trn_rl_repo" not in sys.path:
    sys.path.insert(0, "/opt/trn_rl_repo")

import concourse.bacc as bacc
import concourse.bass as bass
import concourse.tile as tile
from concourse import mybir
from concourse.bass_utils import run_bass_kernel_spmd
from concourse.masks import make_identity

B, S, D, H, DK = 2, 4096, 512, 8, 64
DFF = 2048
EPS = 1e-5
N_CORES = 8
SHARD = S // 4  # 1024 query rows per core
F32 = mybir.dt.float32
F32R = mybir.dt.float32r
AF = mybir.ActivationFunctionType
ALU = mybir.AluOpType

NSLICE = S // 512        # 8 column slices of x^T
NPAIR = H // 2           # 4 head pairs
NKT = S // 128           # 32 key tiles
NQT = SHARD // 128       # 8 query tiles in the shard
NDC = D // 128           # 4 contraction chunks of D
NFC = DFF // 128         # 16 chunks of DFF


def r(ap):
    """View an fp32 AP as float32r for full-rate PE matmuls."""
    return ap.bitcast(F32R)


def _build_program(apply_affine1, apply_affine2, apply_b2, dbg=False):
    nc = bacc.Bacc("TRN2", target_bir_lowering=False, debug=False,
                   num_devices=N_CORES)

    xf = nc.declare_dram_parameter("xf", [S, D], F32, isOutput=False)
    xs = nc.declare_dram_parameter("xs", [SHARD, D], F32, isOutput=False)
    mb = nc.declare_dram_parameter("mb", [S], F32, isOutput=False)
    wq = nc.declare_dram_parameter("wq", [D, D], F32, isOutput=False)
    wk = nc.declare_dram_parameter("wk", [D, D], F32, isOutput=False)
    wv = nc.declare_dram_parameter("wv", [D, D], F32, isOutput=False)
    wo = nc.declare_dram_parameter("wo", [D, D], F32, isOutput=False)
    w1 = nc.declare_dram_parameter("w1", [D, DFF], F32, isOutput=False)
    b1 = nc.declare_dram_parameter("b1", [DFF], F32, isOutput=False)
    w2 = nc.declare_dram_parameter("w2", [DFF, D], F32, isOutput=False)
    b2 = nc.declare_dram_parameter("b2", [D], F32, isOutput=False)
    g1 = nc.declare_dram_parameter("g1", [D], F32, isOutput=False)
    be1 = nc.declare_dram_parameter("be1", [D], F32, isOutput=False)
    g2 = nc.declare_dram_parameter("g2", [D], F32, isOutput=False)
    be2 = nc.declare_dram_parameter("be2", [D], F32, isOutput=False)
    out = nc.declare_dram_parameter("out", [SHARD, D], F32, isOutput=True)

    # DRAM scratch: K^T pair-packed [2*DK rows, S] per pair; raw ctx^T
    kts = [nc.dram_tensor(f"kts{p}", [128, S], F32) for p in range(NPAIR)]
    ctxd = nc.dram_tensor("ctxd", [DK + 1, H, SHARD], F32)
    if dbg:
        dbg_k = nc.declare_dram_parameter("dbg_k", [NPAIR, 128, S], F32, isOutput=True)
        dbg_c = nc.declare_dram_parameter("dbg_c", [64, H, SHARD], F32, isOutput=True)
        dbg_v = nc.declare_dram_parameter("dbg_v", [128, NKT, H, DK + 1], F32, isOutput=True)
        dbg_q = nc.declare_dram_parameter("dbg_q", [128, NPAIR, SHARD], F32, isOutput=True)
        dbg_p = nc.declare_dram_parameter("dbg_p", [128, SHARD], F32, isOutput=True)
        dbg_x1 = nc.declare_dram_parameter("dbg_x1", [SHARD, D], F32, isOutput=True)
        dbg_cr = nc.declare_dram_parameter("dbg_cr", [DK + 1, SHARD], F32, isOutput=True)
        dbg_rb = nc.declare_dram_parameter("dbg_rb", [64, SHARD], F32, isOutput=True)

    def bcast_ap(vec, parts):
        a = vec if isinstance(vec, bass.AP) else vec.ap()
        ap_dims = [list(d) for d in a.ap]
        if len(ap_dims) > 1 and ap_dims[0][1] == 1:
            ap_dims = ap_dims[1:]  # drop a leading singleton partition dim
        return bass.AP(tensor=a.tensor, offset=a.offset,
                       ap=[[0, parts]] + ap_dims)

    with tile.TileContext(nc, pool_alloc_mode="queue") as tc, \
         __import__("contextlib").ExitStack() as ctx:
        consts = ctx.enter_context(tc.tile_pool(name="consts", bufs=1))
        ident = consts.tile([128, 128], F32)
        make_identity(nc, ident)
        mbias = consts.tile([128, NKT], F32)
        nc.sync.dma_start(out=mbias, in_=mb.ap().rearrange("(t p) -> p t", p=128))
        epst = consts.tile([128, 1], F32)
        nc.vector.memset(epst, EPS)

        # ---- resident tensors for phases 1-2 --------------------------
        import contextlib
        es_attn = contextlib.ExitStack()   # v1, qT: phases 1-2
        attn_pool = es_attn.enter_context(tc.tile_pool(name="attn", bufs=1))
        # V with ones column: [p, ktile, head, 65]
        v1 = attn_pool.tile([128, NKT, H, DK + 1], F32R)
        nc.vector.memset(v1[:, :, :, DK:DK + 1].bitcast(F32), 1.0)
        qT = attn_pool.tile([128, NPAIR, SHARD], F32R)

        # ---- phase 1: projections -------------------------------------
        with tc.tile_pool(name="pw", bufs=1) as pw, \
             tc.tile_pool(name="p1s", bufs=3) as p1s, \
             tc.tile_pool(name="p1x", bufs=2) as p1x, \
             tc.tile_pool(name="p1p", bufs=2, space="PSUM") as p1p, \
             tc.tile_pool(name="p1tp", bufs=4, space="PSUM") as p1tp:
            wq_sb = pw.tile([128, NDC, D], F32R)
            nc.sync.dma_start(out=wq_sb, in_=r(wq.ap().rearrange("(c p) n -> p c n", p=128)))
            wk_sb = pw.tile([128, NDC, D], F32R)
            nc.sync.dma_start(out=wk_sb, in_=r(wk.ap().rearrange("(c p) n -> p c n", p=128)))
            wv_sb = pw.tile([128, NDC, D], F32R)
            nc.sync.dma_start(out=wv_sb, in_=r(wv.ap().rearrange("(c p) n -> p c n", p=128)))

            def xT_slice(src, sl, pool, ppool):
                """Load 512 rows of src and return their transpose [128, NDC, 512]."""
                xT = pool.tile([128, NDC, 512], F32R, tag="xTs")
                for m in range(4):
                    xt = p1s.tile([128, D], F32, tag="xload")
                    nc.sync.dma_start(out=xt, in_=src[sl * 512 + m * 128:
                                                      sl * 512 + (m + 1) * 128, :])
                    for c in range(NDC):
                        tp = ppool.tile([128, 128], F32, tag="tpp")
                        nc.tensor.transpose(tp, xt[:, c * 128:(c + 1) * 128], ident)
                        nc.vector.tensor_copy(out=xT[:, c, m * 128:(m + 1) * 128],
                                              in_=tp)
                return xT

            for sl in range(NSLICE):
                xT = xT_slice(xf.ap(), sl, p1x, p1tp)
                # K^T: pair-packed rows, spill to DRAM
                for pr in range(NPAIR):
                    kp = p1p.tile([128, 512], F32, tag="kpsum")
                    for c in range(NDC):
                        nc.tensor.matmul(kp, wk_sb[:, c, pr * 128:(pr + 1) * 128],
                                         xT[:, c, :], start=(c == 0),
                                         stop=(c == NDC - 1))
                    ks = p1s.tile([128, 512], F32, tag="kstage")
                    nc.scalar.copy(out=ks, in_=kp)
                    nc.sync.dma_start(out=kts[pr][:, sl * 512:(sl + 1) * 512], in_=ks)
                # V: natural [s, hdk] layout into the padded ones-column tile
                for m in range(4):
                    vp = p1p.tile([128, 512], F32, tag="vpsum")
                    for c in range(NDC):
                        nc.tensor.matmul(vp, xT[:, c, m * 128:(m + 1) * 128],
                                         wv_sb[:, c, :], start=(c == 0),
                                         stop=(c == NDC - 1))
                    nc.scalar.copy(out=v1[:, sl * 4 + m, :, 0:DK], in_=vp)
            # Q^T from the shard rows (scale folded into wq host-side)
            for sl in range(SHARD // 512):
                xTq = xT_slice(xs.ap(), sl, p1x, p1tp)
                for pr in range(NPAIR):
                    qp = p1p.tile([128, 512], F32, tag="kpsum")
                    for c in range(NDC):
                        nc.tensor.matmul(qp, wq_sb[:, c, pr * 128:(pr + 1) * 128],
                                         xTq[:, c, :], start=(c == 0),
                                         stop=(c == NDC - 1))
                    nc.scalar.copy(out=qT[:, pr, sl * 512:(sl + 1) * 512],
                                   in_=qp)

        if dbg:
            nc.sync.dma_start(out=dbg_v.ap(), in_=v1.bitcast(F32))
            nc.sync.dma_start(out=dbg_q.ap(), in_=qT.bitcast(F32))
            for _p in range(NPAIR):
                nc.sync.dma_start(out=dbg_k[_p, :, :], in_=kts[_p].ap())

        # ---- phase 2: attention ---------------------------------------
        with tc.tile_pool(name="kpool", bufs=2) as kpool, \
             tc.tile_pool(name="ppool", bufs=6) as ppool, \
             tc.tile_pool(name="rpool", bufs=2) as rpool, \
             tc.tile_pool(name="spsum", bufs=2, space="PSUM") as spsum, \
             tc.tile_pool(name="cpsum", bufs=2, space="PSUM") as cpsum:
            for pr in range(NPAIR):
                kT = kpool.tile([128, S], F32R)
                nc.sync.dma_start(out=kT, in_=r(kts[pr][:, :]))
                cA = cpsum.tile([DK + 1, SHARD], F32, tag="ctx")
                cB = cpsum.tile([DK + 1, SHARD], F32, tag="ctx")
                for kt in range(NKT):
                    ksl = kT[:, kt * 128:(kt + 1) * 128]
                    # Each psum tile holds one query-half for BOTH heads:
                    # cols 0:512 = head A, 512:1024 = head B. The A and B
                    # score matmuls are adjacent and use disjoint PE row
                    # groups (0-63 vs 64-127), so they run concurrently.
                    pTs = []
                    for qh in range(SHARD // 512):
                        sp = spsum.tile([128, SHARD], F32, tag="scores")
                        for hh in (0, 1):
                            lo, hi = hh * 64, hh * 64 + 64
                            nc.tensor.matmul(
                                sp[:, hh * 512:(hh + 1) * 512],
                                ksl[lo:hi, :],
                                qT[lo:hi, pr, qh * 512:(qh + 1) * 512],
                                start=True, stop=True)
                        pT = ppool.tile([128, SHARD], F32R, tag="pT")
                        nc.scalar.activation(pT, sp, AF.Exp,
                                             bias=mbias[:, kt:kt + 1], scale=1.0)
                        pTs.append(pT)
                        if dbg and pr == 0 and kt == 0 and qh == 0:
                            nc.sync.dma_start(out=dbg_p.ap(), in_=pT.bitcast(F32))
                    for hh, cps in ((0, cA), (1, cB)):
                        for qh in range(SHARD // 512):
                            nc.tensor.matmul(
                                cps[:, qh * 512:(qh + 1) * 512],
                                v1[:, kt, 2 * pr + hh, :],
                                pTs[qh][:, hh * 512:(hh + 1) * 512],
                                start=(kt == 0), stop=(kt == NKT - 1))
                # spill raw ctx^T + reciprocal rowsum; normalization is
                # applied in phase 3 (keeps the PE/ACT pipeline hot here)
                for hh, cps in ((0, cA), (1, cB)):
                    h = 2 * pr + hh
                    cn = rpool.tile([DK + 1, SHARD], F32, tag="cn")
                    nc.vector.tensor_copy(out=cn, in_=cps)
                    if dbg and pr == 0 and hh == 0:
                        nc.sync.dma_start(out=dbg_cr.ap(), in_=cn)
                    nc.sync.dma_start(out=ctxd[:, h, :], in_=cn)
        es_attn.close()  # free v1 + qT before the FFN residents land
        if dbg:
            nc.sync.dma_start(out=dbg_c.ap(), in_=ctxd[0:DK, :, :])

        # ---- phase 3: attn_out + LN1 + x1^T ---------------------------
        ln1 = ctx.enter_context(tc.tile_pool(name="ln1", bufs=1))
        x1T = ln1.tile([128, NDC, SHARD], F32R)
        x1keep = ln1.tile([128, NQT, D], F32)
        with tc.tile_pool(name="p3w", bufs=1) as p3w, \
             tc.tile_pool(name="p3s", bufs=3) as p3s, \
             tc.tile_pool(name="p3st", bufs=4) as p3st, \
             tc.tile_pool(name="p3p", bufs=2, space="PSUM") as p3p, \
             tc.tile_pool(name="p3tp", bufs=4, space="PSUM") as p3tp:
            wo_sb = p3w.tile([64, H, D], F32R)
            nc.sync.dma_start(out=wo_sb, in_=r(wo.ap().rearrange("(h p) n -> p h n", p=64)))
            if apply_affine1:
                g1b = p3w.tile([128, D], F32)
                nc.sync.dma_start(out=g1b, in_=bcast_ap(g1, 128))
                be1b = p3w.tile([128, D], F32)
                nc.sync.dma_start(out=be1b, in_=bcast_ap(be1, 128))
            cap0 = ctxd.ap()
            for m in range(NQT):
                ctx_m = p3s.tile([64, H, 128], F32, tag="ctxm")
                nc.sync.dma_start(out=ctx_m,
                                  in_=ctxd[0:DK, :, m * 128:(m + 1) * 128])
                # 1/rowsum broadcast across the 64 dk partitions
                rb_m = p3s.tile([64, H, 128], F32, tag="rbm")
                rb_in = bass.AP(
                    tensor=cap0.tensor,
                    offset=cap0.offset + DK * H * SHARD + m * 128,
                    ap=[[0, 64], [SHARD, H], [1, 128]])
                nc.gpsimd.dma_start(out=rb_m, in_=rb_in)
                nc.vector.reciprocal(out=rb_m, in_=rb_m)
                ctx_mn = p3s.tile([64, H, 128], F32R, tag="ctxmn")
                nc.vector.tensor_mul(out=ctx_mn, in0=ctx_m, in1=rb_m)
                ap_ = p3p.tile([128, D], F32, tag="apsum")
                for h in range(H):
                    nc.tensor.matmul(ap_, ctx_mn[:, h, :],
                                     wo_sb[:, h, :], start=(h == 0),
                                     stop=(h == H - 1))
                xt = p3s.tile([128, D], F32, tag="xres")
                nc.sync.dma_start(out=xt, in_=xs[m * 128:(m + 1) * 128, :])
                t = p3s.tile([128, D], F32, tag="tres")
                nc.vector.tensor_add(out=t, in0=ap_, in1=xt)
                stats = p3st.tile([128, 6], F32, tag="stats")
                nc.vector.bn_stats(out=stats, in_=t)
                mv = p3st.tile([128, 2], F32, tag="mv")
                nc.vector.bn_aggr(out=mv, in_=stats)
                sd = p3st.tile([128, 1], F32, tag="sd")
                nc.scalar.activation(out=sd, in_=mv[:, 1:2], func=AF.Sqrt,
                                     bias=epst, scale=1.0)
                rs = p3st.tile([128, 1], F32, tag="rs")
                nc.vector.reciprocal(out=rs, in_=sd)
                x1m = x1keep[:, m, :]
                nc.vector.tensor_scalar(out=x1m, in0=t, scalar1=mv[:, 0:1],
                                        scalar2=rs, op0=ALU.subtract, op1=ALU.mult)
                if apply_affine1:
                    nc.vector.tensor_mul(out=x1m, in0=x1m, in1=g1b)
                    nc.vector.tensor_add(out=x1m, in0=x1m, in1=be1b)
                if dbg:
                    nc.sync.dma_start(out=dbg_x1[m * 128:(m + 1) * 128, :], in_=x1m)
                for c in range(NDC):
                    tp = p3tp.tile([128, 128], F32, tag="tp3")
                    nc.tensor.transpose(tp, x1m[:, c * 128:(c + 1) * 128], ident)
                    nc.scalar.copy(out=x1T[:, c, m * 128:(m + 1) * 128], in_=tp)

        # ---- phase 4: FFN h1^T = relu(w1^T @ x1^T + b1) ---------------
        ffn = ctx.enter_context(tc.tile_pool(name="ffn", bufs=1))
        h1T = ffn.tile([128, NFC, SHARD], F32R)
        with tc.tile_pool(name="p4w", bufs=1) as p4w, \
             tc.tile_pool(name="p4p", bufs=3, space="PSUM") as p4p:
            w1_sb = p4w.tile([128, NDC, DFF], F32R)
            nc.sync.dma_start(out=w1_sb, in_=r(w1.ap().rearrange("(c p) n -> p c n", p=128)))
            b1_sb = p4w.tile([128, NFC], F32)
            nc.sync.dma_start(out=b1_sb, in_=b1.ap().rearrange("(f p) -> p f", p=128))
            for qh in range(SHARD // 512):
                for f in range(NFC):
                    hp = p4p.tile([128, 512], F32, tag="hpsum")
                    for c in range(NDC):
                        nc.tensor.matmul(hp,
                                         w1_sb[:, c, f * 128:(f + 1) * 128],
                                         x1T[:, c, qh * 512:(qh + 1) * 512],
                                         start=(c == 0), stop=(c == NDC - 1))
                    nc.scalar.activation(out=h1T[:, f, qh * 512:(qh + 1) * 512],
                                         in_=hp, func=AF.Relu,
                                         bias=b1_sb[:, f:f + 1], scale=1.0)

        # ---- phase 5: FFN2 + LN2 + output -----------------------------
        with tc.tile_pool(name="p5w", bufs=1) as p5w, \
             tc.tile_pool(name="p5s", bufs=3) as p5s, \
             tc.tile_pool(name="p5st", bufs=4) as p5st, \
             tc.tile_pool(name="p5p", bufs=2, space="PSUM") as p5p:
            w2_sb = p5w.tile([128, NFC, D], F32R)
            nc.sync.dma_start(out=w2_sb, in_=r(w2.ap().rearrange("(f p) n -> p f n", p=128)))
            if apply_b2:
                b2b = p5w.tile([128, D], F32)
                nc.sync.dma_start(out=b2b, in_=bcast_ap(b2, 128))
            if apply_affine2:
                g2b = p5w.tile([128, D], F32)
                nc.sync.dma_start(out=g2b, in_=bcast_ap(g2, 128))
                be2b = p5w.tile([128, D], F32)
                nc.sync.dma_start(out=be2b, in_=bcast_ap(be2, 128))
            for m in range(NQT):
                fp = p5p.tile([128, D], F32, tag="fpsum")
                for f in range(NFC):
                    nc.tensor.matmul(fp, h1T[:, f, m * 128:(m + 1) * 128],
                                     w2_sb[:, f, :], start=(f == 0),
                                     stop=(f == NFC - 1))
                t2 = p5s.tile([128, D], F32, tag="t2")
                nc.vector.tensor_add(out=t2, in0=fp, in1=x1keep[:, m, :])
                if apply_b2:
                    nc.vector.tensor_add(out=t2, in0=t2, in1=b2b)
                stats = p5st.tile([128, 6], F32, tag="stats5")
                nc.vector.bn_stats(out=stats, in_=t2)
                mv = p5st.tile([128, 2], F32, tag="mv5")
                nc.vector.bn_aggr(out=mv, in_=stats)
                sd = p5st.tile([128, 1], F32, tag="sd5")
                nc.scalar.activation(out=sd, in_=mv[:, 1:2], func=AF.Sqrt,
                                     bias=epst, scale=1.0)
                rs = p5st.tile([128, 1], F32, tag="rs5")
                nc.vector.reciprocal(out=rs, in_=sd)
                o = p5s.tile([128, D], F32, tag="otile")
                nc.vector.tensor_scalar(out=o, in0=t2, scalar1=mv[:, 0:1],
                                        scalar2=rs, op0=ALU.subtract, op1=ALU.mult)
                if apply_affine2:
                    nc.vector.tensor_mul(out=o, in0=o, in1=g2b)
                    nc.vector.tensor_add(out=o, in0=o, in1=be2b)
                nc.sync.dma_start(out=out[m * 128:(m + 1) * 128, :], in_=o)

    nc.compile()
    return nc


_PROG_CACHE = {}


def _get_program(key):
    if key not in _PROG_CACHE:
        _PROG_CACHE[key] = _build_program(*key)
    return _PROG_CACHE[key]


def _make_in_maps(x, mask, wq, wk, wv, wo, w1, b1, w2, b2, g1, be1, g2, be2):
    f = np.float32
    wq_s = (wq / np.sqrt(DK)).astype(f)
    mbias = np.where(np.asarray(mask)[:, 0, 0, :] == 0, f(-1e9), f(0.0))
    shared = dict(
        wq=wq_s, wk=np.ascontiguousarray(wk, f), wv=np.ascontiguousarray(wv, f),
        wo=np.ascontiguousarray(wo, f), w1=np.ascontiguousarray(w1, f),
        b1=np.ascontiguousarray(b1, f), w2=np.ascontiguousarray(w2, f),
        b2=np.ascontiguousarray(b2, f), g1=np.ascontiguousarray(g1, f),
        be1=np.ascontiguousarray(be1, f), g2=np.ascontiguousarray(g2, f),
        be2=np.ascontiguousarray(be2, f),
    )
    in_maps = []
    for c in range(N_CORES):
        b, sh = c // 4, c % 4
        m = dict(shared)
        m["xf"] = np.ascontiguousarray(x[b], f)
        m["xs"] = np.ascontiguousarray(x[b, sh * SHARD:(sh + 1) * SHARD], f)
        m["mb"] = np.ascontiguousarray(mbias[b], f)
        in_maps.append(m)
    return in_maps


def kernel(x, mask, wq, wk, wv, wo, w1, b1, w2, b2, g1, be1, g2, be2,
           _trace=False, _tmpdir=None):
    key = (
        not (np.all(g1 == 1.0) and np.all(be1 == 0.0)),
        not (np.all(g2 == 1.0) and np.all(be2 == 0.0)),
        not np.all(b2 == 0.0),
    )
    nc = _get_program(key)
    in_maps = _make_in_maps(x, mask, wq, wk, wv, wo, w1, b1, w2, b2,
                            g1, be1, g2, be2)
    res = None
    for attempt in range(3):
        try:
            res = run_bass_kernel_spmd(nc, in_maps, list(range(N_CORES)),
                                       trace=_trace, tmpdir=_tmpdir)
            break
        except Exception:
            if attempt == 2:
                raise
            import time as _time
            _time.sleep(2.0)
    outs = [res.results[c]["out"] for c in range(N_CORES)]
    full = np.empty((B, S, D), np.float32)
    for c in range(N_CORES):
        b, sh = c // 4, c % 4
        full[b, sh * SHARD:(sh + 1) * SHARD] = outs[c]
    kernel._last_results = res
    return full

